# revision 17
# baseline (speedup 1.0000x reference)
"""Masked dot-product attention on 8 Trainium2 NeuronCores.

Problem shapes (hardcoded): queries/keys/values [128, 1024, 64] f32,
valid_lens [8] int (per-batch key valid length; BH = 8 batches x 16 heads).

Sharding: core c handles heads {b*16 + 2c, b*16 + 2c + 1} for all batches b
(16 heads/core, every batch present on every core -> uniform work, one
compiled program serves all cores).

Layout (host prep, all attention math on device):
  - Q^T [BH, 64, 1024] bf16, PRE-SCALED by G = 0.125*log2(e)*128 so device
    scores arrive as y0 = s*G (the Schraudolph exponent scale). Columns are
    4-way-paired: column cblk*128 + p holds query q = (cblk//4)*512 + 4p +
    (cblk%4), so each PV output subtile maps to >=512B-contiguous DMA runs.
  - K^T [BH, 64, 1024] bf16 natural order (valid-len truncation at 128-chunk
    granularity).
  - V augmented with a ones column (softmax-denominator trick), partition-
    major: [BH, 128, 8, 65] bf16.
  - mask biases [128, 2, b, c]: variant 0 for ACT (0 / -1e6), variant 1 for
    DVE (m*log2e*128 + B0, i.e. the Schraudolph integer bias).

Per-head device pipeline (scores transposed, S^T[k, q]):
  per k-chunk c (only chunks below the batch's valid_len):
    y0^T[c] [128, 1024] = K^T_c.T @ Q^T_scaled      (PSUM f32, 2 bf16 matmuls)
    exp split by static schedule:
      ACT: P^T = exp(y0*SCALE_ACT + mask)  -> bf16  (exact)
      DVE: P^T = bitcast_bf16(int16(max(y0 + maskbias, -32000)))
           (one tensor_scalar; Schraudolph exp2 bit-trick, ~3% max rel err
            on ~40% of chunks -> ~1.1e-2 end-to-end vs 2e-2 budget)
  PV flipped: for each 128-column subtile j: po[:, j, :] [128q, 65] +=
    P^T[c, jslice].T @ [V|1]_c  (stationary P^T, moving V: 65-cycle matmuls,
    full 128-partition output occupancy, no back-transposes needed).
  finalize (DVE): rc = 1/po[:, :, 64]; fin = po[:, :, 0:64]*rc -> bf16;
  DMA out [128, 8, 64] (1KB/partition contiguous); host un-permutes q.

Fully-masked batches (valid_len == 0) are patched on host to the
reference's uniform-softmax value.
"""

import numpy as np
import ml_dtypes

BF16 = ml_dtypes.bfloat16

P = 128          # partitions / k-chunk size
D = 64           # head dim
QL = 1024        # query length
KL = 1024        # key length
NB = 8           # batches
NH = 16          # heads per batch
NCORES = 8
HPC = 16         # heads per core
NCHUNK = KL // P # 8 k-chunks
NEG = -1.0e6

G = 0.125 * np.log2(np.e) * 128.0        # Q pre-scale (Schraudolph exponent)
SCALE_ACT = float(0.125 / G)             # ACT: exp(y0*SCALE_ACT + mask)
MBIAS = float(np.log2(np.e) * 128.0)     # mask multiplier for DVE bias
B0 = 16248.5                             # Schraudolph magic (nearest rounding)
CLAMP = -32000.0                         # masked lanes -> int16 -> bf16 ~ -0

# exp engine schedule: 'A' = ACT exact exp, 'D' = DVE Schraudolph
PATTERN = "ADADADADADAA"
_WARMUP = 8


def _split_excess_waits(nc, max_waits=1):
    """This walrus (gen3) accepts only one sync-wait per instruction, but Tile
    emits up to 2 on compute ops and 5+ on the kernel-tail drain. Hoist excess
    on_wait entries onto fresh InstEventSemaphore ops on the same engine,
    inserted immediately before the offending instruction (same semantics:
    the engine stalls on each wait sequentially)."""
    import bass_rust
    import concourse.mybir as mybir

    n_split = 0
    for func in nc.m.functions:
        for block in func.blocks:
            out = []
            changed = False
            for inst in block.instructions:
                si = getattr(inst, "sync_info", None)
                waits = list(si.on_wait) if si is not None else []
                if len(waits) > max_waits:
                    changed = True
                    for w in waits[:-max_waits]:
                        n_split += 1
                        out.append(
                            mybir.InstEventSemaphore(
                                name=f"waitsplit_{n_split}_{inst.name}",
                                engine=inst.engine,
                                ins=[],
                                outs=[],
                                sync_info=bass_rust.SyncInfo(
                                    on_wait=[w], on_update=[]
                                ),
                            )
                        )
                    inst.sync_info = bass_rust.SyncInfo(
                        on_wait=waits[-max_waits:], on_update=list(si.on_update)
                    )
                out.append(inst)
            if changed:
                block.instructions = out
    return n_split


def _build(nc_chunks=None):
    import concourse.bass as bass
    import concourse.mybir as mybir
    from concourse.tile import TileContext

    if nc_chunks is None:
        nc_chunks = [NCHUNK] * NB

    f32 = mybir.dt.float32
    bf16 = mybir.dt.bfloat16
    i16 = mybir.dt.int16
    Exp = mybir.ActivationFunctionType.Exp
    Add = mybir.AluOpType.add
    Max = mybir.AluOpType.max

    nc = bass.Bass(trn_type="TRN2")
    qkd = nc.dram_tensor("qk", [HPC, D, QL + KL], bf16, kind="ExternalInput")
    vd = nc.dram_tensor("v", [HPC, P, NCHUNK, D + 1], bf16, kind="ExternalInput")
    md = nc.dram_tensor("mask", [P, 2 * NB * NCHUNK], f32, kind="ExternalInput")
    od = nc.dram_tensor("out", [HPC, P, NCHUNK, D], bf16, kind="ExternalOutput")

    gidx = [0]  # global exp-chunk counter for the A/D schedule

    with TileContext(nc) as tc:
        with (
            tc.tile_pool(name="consts", bufs=1) as consts,
            tc.tile_pool(name="io", bufs=3) as io,
            tc.tile_pool(name="pt", bufs=2) as ptp,
            tc.tile_pool(name="fin", bufs=2) as finp,
            tc.tile_pool(name="rc", bufs=2) as rcp,
            tc.tile_pool(name="ps_s", bufs=3, space="PSUM") as ps_s,
            tc.tile_pool(name="ps_o", bufs=1, space="PSUM") as ps_o,
        ):
            # preamble ordering matters: the Pool queue runs the tiny
            # memsets BEFORE the mask SWDGE load (so the PE warmups start
            # early), and the first head's qk DMA goes on the ACT ring
            # BEFORE the exp-table priming activation (so it isn't stuck
            # behind the table load)
            mask_sb = consts.tile([P, 2, NB, NCHUNK], f32)
            scratch = consts.tile([1, 1], f32)
            warm_sb = consts.tile([P, 256], bf16)
            nc.gpsimd.memset(scratch, 0.0)
            nc.gpsimd.memset(warm_sb, 0.0)
            nc.gpsimd.dma_start(
                out=mask_sb, in_=md.rearrange("p (t b c) -> p t b c", t=2, b=NB)
            )

            def emit_front(h, first=False):
                b = h // 2
                nck = nc_chunks[b]
                qk = io.tile([D, QL + KL], bf16, tag="qk")
                qt = qk[:, 0:QL]
                kt = qk[:, QL : QL + KL]
                if first:
                    # the first score matmul needs qt halves + kt chunk 0;
                    # one contiguous load covers qt plus kt chunk 0, a second
                    # brings the rest (the HWDGE processes descriptors
                    # serially, so fewer/earlier descriptors win)
                    nc.scalar.dma_start(
                        out=qk[:, 0 : QL + P], in_=qkd[h][:, 0 : QL + P]
                    )
                    nc.sync.dma_start(
                        out=qk[:, QL + P :], in_=qkd[h][:, QL + P :]
                    )
                else:
                    nc.sync.dma_start(
                        out=qk[:, 0 : QL + nck * P],
                        in_=qkd[h][:, 0 : QL + nck * P],
                    )
                v1 = io.tile([P, NCHUNK, D + 1], bf16, tag="v")
                nc.sync.dma_start(
                    out=v1[:, 0:nck, :], in_=vd[h][:, 0:nck, :]
                )
                return qt, kt, v1

            def emit_score_chunk(h, state, pt, c, force_kind=None):
                b = h // 2
                qt, kt, v1 = state
                ps = ps_s.tile([P, QL], f32, tag="s")
                nc.tensor.matmul(
                    ps[:, 0:512],
                    kt[:, c * P : (c + 1) * P],
                    qt[:, 0:512],
                    start=True, stop=True,
                )
                nc.tensor.matmul(
                    ps[:, 512:QL],
                    kt[:, c * P : (c + 1) * P],
                    qt[:, 512:QL],
                    start=True, stop=True,
                )
                kind = PATTERN[gidx[0] % len(PATTERN)] if force_kind is None else force_kind
                gidx[0] += 1
                if kind == "A":
                    nc.scalar.activation(
                        pt[:, c, :], ps, Exp,
                        bias=mask_sb[:, 0, b, c : c + 1], scale=SCALE_ACT,
                    )
                else:
                    pt_i16 = pt.bitcast(i16)
                    nc.vector.tensor_scalar(
                        pt_i16[:, c, :], ps,
                        mask_sb[:, 1, b, c : c + 1], CLAMP,
                        op0=Add, op1=Max,
                    )

            def emit_pv_group(h, po, pt, v1, j, nck):
                # one output subtile j: consecutive accumulating matmuls
                # (interleaved PSUM accumulation groups don't accumulate
                # correctly, so keep each group's matmuls back-to-back)
                for c in range(nck):
                    nc.tensor.matmul(
                        po[:, j, 0 : D + 1],
                        pt[:, c, j * P : (j + 1) * P],
                        v1[:, c, :],
                        start=(c == 0), stop=(c == nck - 1),
                    )

            def emit_fin(h, po, split=False):
                fin = finp.tile([P, NCHUNK, D], bf16, tag="fin")
                H = NCHUNK // 2
                for lo, hi in ([(0, H), (H, NCHUNK)] if split else [(0, NCHUNK)]):
                    n = hi - lo
                    rc = rcp.tile([P, NCHUNK], f32, tag="rc")
                    nc.vector.reciprocal(
                        rc[:, 0:n], po[:, lo:hi, D : D + 1]
                    )
                    nc.vector.tensor_mul(
                        fin[:, lo:hi],
                        po[:, lo:hi, 0:D],
                        rc[:, 0:n, None].broadcast_to([P, n, D]),
                    )
                    nc.sync.dma_start(out=od[h][:, lo:hi], in_=fin[:, lo:hi])

            def emit_iter(cur, prev, last=False):
                """cur = (h, state): scores+exp; prev = (h, state, pt):
                PV + finalize, interleaved chunk-wise on the PE. PV group
                emission is delayed by one chunk so the previous head's
                finalize (which frees the single po buffer) clears first."""
                pt_cur = None
                if cur is not None:
                    h, state = cur
                    nck = nc_chunks[h // 2]
                    pt_cur = ptp.tile([P, NCHUNK, QL], bf16, tag="pt")
                po = None
                if prev is not None:
                    ph, pstate, ppt = prev
                    pnck = nc_chunks[ph // 2]
                    # [P, 8, 128] f32: 512B-aligned j-subtiles (a matmul
                    # output must not cross a 2KB PSUM bank boundary). The
                    # tail iteration takes a ps_s slot instead: no score
                    # tiles compete then, and it skips the po-buffer WAR
                    # against the previous finalize.
                    if last:
                        po = ps_s.tile([P, NCHUNK, P], f32, tag="s")
                    else:
                        po = ps_o.tile([P, NCHUNK, P], f32, tag="o")
                n_s = nck if cur is not None else 0
                steps = max(n_s, 1)
                for c in range(steps):
                    if c < n_s:
                        # the final head's exps alternate engines so they
                        # run in parallel rather than queueing on one
                        emit_score_chunk(
                            h, state, pt_cur, c,
                            force_kind=("DA"[c % 2] if last else None),
                        )
                    if prev is not None:
                        if steps == 1:
                            jlo, jhi = 0, NCHUNK
                        else:
                            jlo = NCHUNK * max(c - 1, 0) // (steps - 1)
                            jhi = NCHUNK * c // (steps - 1)
                        for j in range(jlo, jhi):
                            emit_pv_group(ph, po, ppt, pstate[2], j, pnck)
                if prev is not None:
                    emit_fin(ph, po)
                return pt_cur

            # Interleave big and small heads so engine loads stay balanced;
            # end with the smallest head (shortest un-hidden tail).
            by_size = sorted(range(HPC), key=lambda h: -nc_chunks[h // 2])
            big, small = by_size[: HPC // 2], by_size[HPC // 2 :]
            order = [h for pair in zip(big, small) for h in pair]

            fronts = {}
            fronts[order[0]] = emit_front(order[0], first=True)
            # prime the ScalarE exp table load (~1.3us) and the PE p-state
            # ramp while the first qk DMA is in flight
            nc.scalar.activation(scratch, scratch, Exp)
            warm = ps_s.tile([1, 256], f32, tag="s")
            for _ in range(_WARMUP):
                nc.tensor.matmul(
                    warm, warm_sb[:, 0:1], warm_sb[:, 0:256],
                    start=True, stop=True,
                )
            pending = None   # (h, state) awaiting scores
            prev = None      # (h, state, pt) awaiting pv+fin
            for i, h in enumerate(order):
                if i > 0:
                    fronts[h] = emit_front(h)
                if pending is not None:
                    pt = emit_iter(pending, prev)
                    prev = (pending[0], pending[1], pt)
                pending = (h, fronts[h])
            pt = emit_iter(pending, prev, last=True)
            prev = (pending[0], pending[1], pt)
            emit_iter(None, prev, last=True)
    _split_excess_waits(nc)
    return nc


_CACHE = {}


def _get_nc(key, nc_chunks):
    if key not in _CACHE:
        _CACHE[key] = _build(nc_chunks)
    return _CACHE[key]


def _core_head_idx(c):
    return [b * NH + 2 * c + j for b in range(NB) for j in range(2)]


def _run(in_maps, nc, trace=False):
    from concourse.bass_utils import run_bass_kernel_spmd

    return run_bass_kernel_spmd(
        nc, in_maps, core_ids=list(range(NCORES)), trace=trace
    )


# column cblk*128 + p holds query q = (cblk//4)*512 + 4p + (cblk%4)
_COLQ = np.empty(QL, np.int64)
for _cb in range(8):
    for _p in range(P):
        _COLQ[_cb * P + _p] = (_cb // 4) * 512 + 4 * _p + (_cb % 4)
# output index: od[p, cblk, :] -> q = _COLQ[cblk*128 + p]
_OUTQ = np.empty(QL, np.int64)
for _p in range(P):
    for _cb in range(8):
        _OUTQ[_p * 8 + _cb] = _COLQ[_cb * P + _p]


def _prepare(queries, keys, values, valid_lens):
    queries = np.asarray(queries, np.float32)
    keys = np.asarray(keys, np.float32)
    values = np.asarray(values, np.float32)
    vl = np.asarray(valid_lens).astype(np.int64)
    maskv = np.where(
        np.arange(KL)[None, :] >= vl[:, None], np.float32(NEG), np.float32(0.0)
    ).astype(np.float32)  # [NB, KL]
    # [p, b, c] = mask[b, c*128 + p]
    m_pbc = maskv.reshape(NB, NCHUNK, P).transpose(2, 0, 1)
    mask_dev = np.empty((P, 2, NB, NCHUNK), np.float32)
    mask_dev[:, 0] = m_pbc
    mask_dev[:, 1] = m_pbc * np.float32(MBIAS) + np.float32(B0)
    mask_dev = np.ascontiguousarray(
        mask_dev.reshape(P, 2 * NB * NCHUNK)
    )
    nc_chunks = [max(1, int(min(NCHUNK, (int(v) + P - 1) // P))) for v in vl]
    bh = queries.shape[0]
    # [Q^T | K^T] combined, bf16; Q pre-scaled, 4-paired column order
    qkp = np.empty((bh, D, QL + KL), BF16)
    qkp[:, :, 0:QL] = (queries[:, _COLQ, :] * np.float32(G)).transpose(0, 2, 1)
    qkp[:, :, QL:] = keys.transpose(0, 2, 1)
    # V + ones column, partition-major: [bh, 128, 8, 65]
    v1 = np.concatenate(
        [values, np.ones((bh, KL, 1), np.float32)], axis=-1
    )
    v1p = np.ascontiguousarray(
        v1.reshape(bh, NCHUNK, P, D + 1).transpose(0, 2, 1, 3)
    ).astype(BF16)
    in_maps = []
    for c in range(NCORES):
        idx = _core_head_idx(c)
        in_maps.append(
            {
                "qk": qkp[idx],
                "v": v1p[idx],
                "mask": mask_dev,
            }
        )
    return in_maps, nc_chunks, vl


def _gather(results, values, vl):
    out = np.empty((NB * NH, QL, D), np.float32)
    for c in range(NCORES):
        o = np.asarray(results[c]["out"]).astype(np.float32)  # [16,128,8,64]
        out[_core_head_idx(c)] = _unpermute(o)
    # fully-masked batches: reference softmax(-1e6 * ones) is uniform
    for b in range(NB):
        if vl[b] == 0:
            for hh in range(NH):
                bhh = b * NH + hh
                out[bhh] = np.asarray(values[bhh], np.float32).mean(
                    axis=0, keepdims=True
                )
    return out


def _unpermute(o):
    # o [HPC, 128, 8, 64] -> [HPC, QL, D] with q = _OUTQ[p*8+c]
    flat = o.reshape(HPC, QL, D)
    res = np.empty_like(flat)
    res[:, _OUTQ] = flat
    return res


def kernel(queries, keys, values, valid_lens):
    in_maps, nc_chunks, vl = _prepare(queries, keys, values, valid_lens)
    nc = _get_nc(tuple(nc_chunks), nc_chunks)
    res = _run(in_maps, nc)
    return _gather(res.results, values, vl)


# revision 18
# speedup vs baseline: 1.0585x; 1.0585x over previous
"""Masked dot-product attention on 8 Trainium2 NeuronCores.

Problem shapes (hardcoded): queries/keys/values [128, 1024, 64] f32,
valid_lens [8] int (per-batch key valid length; BH = 8 batches x 16 heads).

Sharding: core c handles heads {b*16 + 2c, b*16 + 2c + 1} for all batches b
(16 heads/core, every batch present on every core -> uniform work, one
compiled program serves all cores).

Layout (host prep, all attention math on device):
  - Q^T [BH, 64, 1024] bf16, PRE-SCALED by G = 0.125*log2(e)*128 so device
    scores arrive as y0 = s*G (the Schraudolph exponent scale). Columns are
    4-way-paired: column cblk*128 + p holds query q = (cblk//4)*512 + 4p +
    (cblk%4), so each PV output subtile maps to >=512B-contiguous DMA runs.
  - K^T [BH, 64, 1024] bf16 natural order (valid-len truncation at 128-chunk
    granularity).
  - V augmented with a ones column (softmax-denominator trick), partition-
    major: [BH, 128, 8, 65] bf16.
  - mask biases [128, 2, b, c]: variant 0 for ACT (0 / -1e6), variant 1 for
    DVE (m*log2e*128 + B0, i.e. the Schraudolph integer bias).

Per-head device pipeline (scores transposed, S^T[k, q]):
  per k-chunk c (only chunks below the batch's valid_len):
    y0^T[c] [128, 1024] = K^T_c.T @ Q^T_scaled      (PSUM f32, 2 bf16 matmuls)
    exp split by static schedule:
      ACT: P^T = exp(y0*SCALE_ACT + mask)  -> bf16  (exact)
      DVE: P^T = bitcast_bf16(int16(max(y0 + maskbias, -32000)))
           (one tensor_scalar; Schraudolph exp2 bit-trick, ~3% max rel err
            on ~40% of chunks -> ~1.1e-2 end-to-end vs 2e-2 budget)
  PV flipped: for each 128-column subtile j: po[:, j, :] [128q, 65] +=
    P^T[c, jslice].T @ [V|1]_c  (stationary P^T, moving V: 65-cycle matmuls,
    full 128-partition output occupancy, no back-transposes needed).
  finalize (DVE): rc = 1/po[:, :, 64]; fin = po[:, :, 0:64]*rc -> bf16;
  DMA out [128, 8, 64] (1KB/partition contiguous); host un-permutes q.

Fully-masked batches (valid_len == 0) are patched on host to the
reference's uniform-softmax value.
"""

import numpy as np
import ml_dtypes

BF16 = ml_dtypes.bfloat16

P = 128          # partitions / k-chunk size
D = 64           # head dim
QL = 1024        # query length
KL = 1024        # key length
NB = 8           # batches
NH = 16          # heads per batch
NCORES = 8
HPC = 16         # heads per core
NCHUNK = KL // P # 8 k-chunks
NEG = -1.0e6

G = 0.125 * np.log2(np.e) * 128.0        # Q pre-scale (Schraudolph exponent)
SCALE_ACT = float(0.125 / G)             # ACT: exp(y0*SCALE_ACT + mask)
MBIAS = float(np.log2(np.e) * 128.0)     # mask multiplier for DVE bias
B0 = 16248.5                             # Schraudolph magic (nearest rounding)
CLAMP = -32000.0                         # masked lanes -> int16 -> bf16 ~ -0

# exp engine schedule: 'A' = ACT exact exp, 'D' = DVE Schraudolph
PATTERN = "ADAADAAD"
_WARMUP = 8


def _split_excess_waits(nc, max_waits=1):
    """This walrus (gen3) accepts only one sync-wait per instruction, but Tile
    emits up to 2 on compute ops and 5+ on the kernel-tail drain. Hoist excess
    on_wait entries onto fresh InstEventSemaphore ops on the same engine,
    inserted immediately before the offending instruction (same semantics:
    the engine stalls on each wait sequentially)."""
    import bass_rust
    import concourse.mybir as mybir

    n_split = 0
    for func in nc.m.functions:
        for block in func.blocks:
            out = []
            changed = False
            for inst in block.instructions:
                si = getattr(inst, "sync_info", None)
                waits = list(si.on_wait) if si is not None else []
                if len(waits) > max_waits:
                    changed = True
                    for w in waits[:-max_waits]:
                        n_split += 1
                        out.append(
                            mybir.InstEventSemaphore(
                                name=f"waitsplit_{n_split}_{inst.name}",
                                engine=inst.engine,
                                ins=[],
                                outs=[],
                                sync_info=bass_rust.SyncInfo(
                                    on_wait=[w], on_update=[]
                                ),
                            )
                        )
                    inst.sync_info = bass_rust.SyncInfo(
                        on_wait=waits[-max_waits:], on_update=list(si.on_update)
                    )
                out.append(inst)
            if changed:
                block.instructions = out
    return n_split


def _build(nc_chunks=None):
    import concourse.bass as bass
    import concourse.mybir as mybir
    from concourse.tile import TileContext

    if nc_chunks is None:
        nc_chunks = [NCHUNK] * NB

    f32 = mybir.dt.float32
    bf16 = mybir.dt.bfloat16
    i16 = mybir.dt.int16
    Exp = mybir.ActivationFunctionType.Exp
    Add = mybir.AluOpType.add
    Max = mybir.AluOpType.max

    nc = bass.Bass(trn_type="TRN2")
    qkd = nc.dram_tensor("qk", [HPC, D, QL + KL], bf16, kind="ExternalInput")
    vd = nc.dram_tensor("v", [HPC, P, NCHUNK, D + 1], bf16, kind="ExternalInput")
    md = nc.dram_tensor("mask", [P, 2 * NB * NCHUNK], f32, kind="ExternalInput")
    od = nc.dram_tensor("out", [HPC, P, NCHUNK, D], bf16, kind="ExternalOutput")

    gidx = [0]  # global exp-chunk counter for the A/D schedule

    with TileContext(nc) as tc:
        with (
            tc.tile_pool(name="consts", bufs=1) as consts,
            tc.tile_pool(name="io", bufs=3) as io,
            tc.tile_pool(name="pt", bufs=2) as ptp,
            tc.tile_pool(name="fin", bufs=2) as finp,
            tc.tile_pool(name="rc", bufs=2) as rcp,
            tc.tile_pool(name="ps_s", bufs=3, space="PSUM") as ps_s,
            tc.tile_pool(name="ps_o", bufs=1, space="PSUM") as ps_o,
        ):
            # preamble ordering matters: the Pool queue runs the tiny
            # memsets BEFORE the mask SWDGE load (so the PE warmups start
            # early), and the first head's qk DMA goes on the ACT ring
            # BEFORE the exp-table priming activation (so it isn't stuck
            # behind the table load)
            mask_sb = consts.tile([P, 2, NB, NCHUNK], f32)
            scratch = consts.tile([1, 1], f32)
            warm_sb = consts.tile([P, 256], bf16)
            nc.gpsimd.memset(scratch, 0.0)
            nc.gpsimd.memset(warm_sb, 0.0)
            nc.gpsimd.dma_start(
                out=mask_sb, in_=md.rearrange("p (t b c) -> p t b c", t=2, b=NB)
            )

            def emit_front(h, first=False):
                b = h // 2
                nck = nc_chunks[b]
                qk = io.tile([D, QL + KL], bf16, tag="qk")
                qt = qk[:, 0:QL]
                kt = qk[:, QL : QL + KL]
                if first:
                    # the first score matmul needs qt halves + kt chunk 0;
                    # one contiguous load covers qt plus kt chunk 0, a second
                    # brings the rest (the HWDGE processes descriptors
                    # serially, so fewer/earlier descriptors win)
                    nc.scalar.dma_start(
                        out=qk[:, 0 : QL + P], in_=qkd[h][:, 0 : QL + P]
                    )
                    nc.sync.dma_start(
                        out=qk[:, QL + P :], in_=qkd[h][:, QL + P :]
                    )
                else:
                    nc.sync.dma_start(
                        out=qk[:, 0 : QL + nck * P],
                        in_=qkd[h][:, 0 : QL + nck * P],
                    )
                v1 = io.tile([P, NCHUNK, D + 1], bf16, tag="v")
                nc.sync.dma_start(
                    out=v1[:, 0:nck, :], in_=vd[h][:, 0:nck, :]
                )
                return qt, kt, v1

            def emit_score_chunk(h, state, pt, c, force_kind=None):
                b = h // 2
                qt, kt, v1 = state
                ps = ps_s.tile([P, QL], f32, tag="s")
                nc.tensor.matmul(
                    ps[:, 0:512],
                    kt[:, c * P : (c + 1) * P],
                    qt[:, 0:512],
                    start=True, stop=True,
                )
                nc.tensor.matmul(
                    ps[:, 512:QL],
                    kt[:, c * P : (c + 1) * P],
                    qt[:, 512:QL],
                    start=True, stop=True,
                )
                kind = PATTERN[gidx[0] % len(PATTERN)] if force_kind is None else force_kind
                gidx[0] += 1
                if kind == "A":
                    nc.scalar.activation(
                        pt[:, c, :], ps, Exp,
                        bias=mask_sb[:, 0, b, c : c + 1], scale=SCALE_ACT,
                    )
                else:
                    pt_i16 = pt.bitcast(i16)
                    nc.vector.tensor_scalar(
                        pt_i16[:, c, :], ps,
                        mask_sb[:, 1, b, c : c + 1], CLAMP,
                        op0=Add, op1=Max,
                    )

            def emit_pv_group(h, po, pt, v1, j, nck):
                # one output subtile j: consecutive accumulating matmuls
                # (interleaved PSUM accumulation groups don't accumulate
                # correctly, so keep each group's matmuls back-to-back)
                for c in range(nck):
                    nc.tensor.matmul(
                        po[:, j, 0 : D + 1],
                        pt[:, c, j * P : (j + 1) * P],
                        v1[:, c, :],
                        start=(c == 0), stop=(c == nck - 1),
                    )

            def emit_fin(h, po, split=False):
                fin = finp.tile([P, NCHUNK, D], bf16, tag="fin")
                H = NCHUNK // 2
                for lo, hi in ([(0, H), (H, NCHUNK)] if split else [(0, NCHUNK)]):
                    n = hi - lo
                    rc = rcp.tile([P, NCHUNK], f32, tag="rc")
                    nc.vector.reciprocal(
                        rc[:, 0:n], po[:, lo:hi, D : D + 1]
                    )
                    nc.vector.tensor_mul(
                        fin[:, lo:hi],
                        po[:, lo:hi, 0:D],
                        rc[:, 0:n, None].broadcast_to([P, n, D]),
                    )
                    nc.sync.dma_start(out=od[h][:, lo:hi], in_=fin[:, lo:hi])

            def emit_iter(cur, prev, last=False):
                """cur = (h, state): scores+exp; prev = (h, state, pt):
                PV + finalize, interleaved chunk-wise on the PE. PV group
                emission is delayed by one chunk so the previous head's
                finalize (which frees the single po buffer) clears first."""
                pt_cur = None
                if cur is not None:
                    h, state = cur
                    nck = nc_chunks[h // 2]
                    pt_cur = ptp.tile([P, NCHUNK, QL], bf16, tag="pt")
                po = None
                if prev is not None:
                    ph, pstate, ppt = prev
                    pnck = nc_chunks[ph // 2]
                    # [P, 8, 128] f32: 512B-aligned j-subtiles (a matmul
                    # output must not cross a 2KB PSUM bank boundary). The
                    # tail iteration takes a ps_s slot instead: no score
                    # tiles compete then, and it skips the po-buffer WAR
                    # against the previous finalize.
                    if last:
                        po = ps_s.tile([P, NCHUNK, P], f32, tag="s")
                    else:
                        po = ps_o.tile([P, NCHUNK, P], f32, tag="o")
                n_s = nck if cur is not None else 0
                steps = max(n_s, 1)
                for c in range(steps):
                    if c < n_s:
                        # the final head's exps alternate engines so they
                        # run in parallel rather than queueing on one
                        emit_score_chunk(
                            h, state, pt_cur, c,
                            force_kind=("DA"[c % 2] if last else None),
                        )
                    if prev is not None:
                        if steps == 1:
                            jlo, jhi = 0, NCHUNK
                        else:
                            jlo = NCHUNK * max(c - 1, 0) // (steps - 1)
                            jhi = NCHUNK * c // (steps - 1)
                        for j in range(jlo, jhi):
                            emit_pv_group(ph, po, ppt, pstate[2], j, pnck)
                if prev is not None:
                    emit_fin(ph, po)
                return pt_cur

            # Interleave big and small heads so engine loads stay balanced;
            # end with the smallest head (shortest un-hidden tail).
            by_size = sorted(range(HPC), key=lambda h: -nc_chunks[h // 2])
            big, small = by_size[: HPC // 2], by_size[HPC // 2 :]
            order = [h for pair in zip(big, small) for h in pair]

            fronts = {}
            fronts[order[0]] = emit_front(order[0], first=True)
            # prime the ScalarE exp table load (~1.3us) and the PE p-state
            # ramp while the first qk DMA is in flight
            nc.scalar.activation(scratch, scratch, Exp)
            warm = ps_s.tile([1, 256], f32, tag="s")
            for _ in range(_WARMUP):
                nc.tensor.matmul(
                    warm, warm_sb[:, 0:1], warm_sb[:, 0:256],
                    start=True, stop=True,
                )
            pending = None   # (h, state) awaiting scores
            prev = None      # (h, state, pt) awaiting pv+fin
            for i, h in enumerate(order):
                if i > 0:
                    fronts[h] = emit_front(h)
                if pending is not None:
                    pt = emit_iter(pending, prev)
                    prev = (pending[0], pending[1], pt)
                pending = (h, fronts[h])
            pt = emit_iter(pending, prev, last=True)
            prev = (pending[0], pending[1], pt)
            emit_iter(None, prev, last=True)
    _split_excess_waits(nc)
    return nc


_CACHE = {}


def _get_nc(key, nc_chunks):
    if key not in _CACHE:
        _CACHE[key] = _build(nc_chunks)
    return _CACHE[key]


def _core_head_idx(c):
    return [b * NH + 2 * c + j for b in range(NB) for j in range(2)]


def _run(in_maps, nc, trace=False):
    from concourse.bass_utils import run_bass_kernel_spmd

    return run_bass_kernel_spmd(
        nc, in_maps, core_ids=list(range(NCORES)), trace=trace
    )


# column cblk*128 + p holds query q = (cblk//4)*512 + 4p + (cblk%4)
_COLQ = np.empty(QL, np.int64)
for _cb in range(8):
    for _p in range(P):
        _COLQ[_cb * P + _p] = (_cb // 4) * 512 + 4 * _p + (_cb % 4)
# output index: od[p, cblk, :] -> q = _COLQ[cblk*128 + p]
_OUTQ = np.empty(QL, np.int64)
for _p in range(P):
    for _cb in range(8):
        _OUTQ[_p * 8 + _cb] = _COLQ[_cb * P + _p]


def _prepare(queries, keys, values, valid_lens):
    queries = np.asarray(queries, np.float32)
    keys = np.asarray(keys, np.float32)
    values = np.asarray(values, np.float32)
    vl = np.asarray(valid_lens).astype(np.int64)
    maskv = np.where(
        np.arange(KL)[None, :] >= vl[:, None], np.float32(NEG), np.float32(0.0)
    ).astype(np.float32)  # [NB, KL]
    # [p, b, c] = mask[b, c*128 + p]
    m_pbc = maskv.reshape(NB, NCHUNK, P).transpose(2, 0, 1)
    mask_dev = np.empty((P, 2, NB, NCHUNK), np.float32)
    mask_dev[:, 0] = m_pbc
    mask_dev[:, 1] = m_pbc * np.float32(MBIAS) + np.float32(B0)
    mask_dev = np.ascontiguousarray(
        mask_dev.reshape(P, 2 * NB * NCHUNK)
    )
    nc_chunks = [max(1, int(min(NCHUNK, (int(v) + P - 1) // P))) for v in vl]
    bh = queries.shape[0]
    # [Q^T | K^T] combined, bf16; Q pre-scaled, 4-paired column order
    qkp = np.empty((bh, D, QL + KL), BF16)
    qkp[:, :, 0:QL] = (queries[:, _COLQ, :] * np.float32(G)).transpose(0, 2, 1)
    qkp[:, :, QL:] = keys.transpose(0, 2, 1)
    # V + ones column, partition-major: [bh, 128, 8, 65]
    v1 = np.concatenate(
        [values, np.ones((bh, KL, 1), np.float32)], axis=-1
    )
    v1p = np.ascontiguousarray(
        v1.reshape(bh, NCHUNK, P, D + 1).transpose(0, 2, 1, 3)
    ).astype(BF16)
    in_maps = []
    for c in range(NCORES):
        idx = _core_head_idx(c)
        in_maps.append(
            {
                "qk": qkp[idx],
                "v": v1p[idx],
                "mask": mask_dev,
            }
        )
    return in_maps, nc_chunks, vl


def _gather(results, values, vl):
    out = np.empty((NB * NH, QL, D), np.float32)
    for c in range(NCORES):
        o = np.asarray(results[c]["out"]).astype(np.float32)  # [16,128,8,64]
        out[_core_head_idx(c)] = _unpermute(o)
    # fully-masked batches: reference softmax(-1e6 * ones) is uniform
    for b in range(NB):
        if vl[b] == 0:
            for hh in range(NH):
                bhh = b * NH + hh
                out[bhh] = np.asarray(values[bhh], np.float32).mean(
                    axis=0, keepdims=True
                )
    return out


def _unpermute(o):
    # o [HPC, 128, 8, 64] -> [HPC, QL, D] with q = _OUTQ[p*8+c]
    flat = o.reshape(HPC, QL, D)
    res = np.empty_like(flat)
    res[:, _OUTQ] = flat
    return res


def kernel(queries, keys, values, valid_lens):
    in_maps, nc_chunks, vl = _prepare(queries, keys, values, valid_lens)
    nc = _get_nc(tuple(nc_chunks), nc_chunks)
    res = _run(in_maps, nc)
    return _gather(res.results, values, vl)


# revision 23
# speedup vs baseline: 1.0649x; 1.0061x over previous
"""Masked dot-product attention on 8 Trainium2 NeuronCores.

Problem shapes (hardcoded): queries/keys/values [128, 1024, 64] f32,
valid_lens [8] int (per-batch key valid length; BH = 8 batches x 16 heads).

Sharding: core c handles heads {b*16 + 2c, b*16 + 2c + 1} for all batches b
(16 heads/core, every batch present on every core -> uniform work, one
compiled program serves all cores).

Layout (host prep, all attention math on device):
  - Q^T [BH, 64, 1024] bf16, PRE-SCALED by G = 0.125*log2(e)*128 so device
    scores arrive as y0 = s*G (the Schraudolph exponent scale). Columns are
    4-way-paired: column cblk*128 + p holds query q = (cblk//4)*512 + 4p +
    (cblk%4), so each PV output subtile maps to >=512B-contiguous DMA runs.
  - K^T [BH, 64, 1024] bf16 natural order (valid-len truncation at 128-chunk
    granularity).
  - V augmented with a ones column (softmax-denominator trick), partition-
    major: [BH, 128, 8, 65] bf16.
  - mask biases [128, 2, b, c]: variant 0 for ACT (0 / -1e6), variant 1 for
    DVE (m*log2e*128 + B0, i.e. the Schraudolph integer bias).

Per-head device pipeline (scores transposed, S^T[k, q]):
  per k-chunk c (only chunks below the batch's valid_len):
    y0^T[c] [128, 1024] = K^T_c.T @ Q^T_scaled      (PSUM f32, 2 bf16 matmuls)
    exp split by static schedule:
      ACT: P^T = exp(y0*SCALE_ACT + mask)  -> bf16  (exact)
      DVE: P^T = bitcast_bf16(int16(max(y0 + maskbias, -32000)))
           (one tensor_scalar; Schraudolph exp2 bit-trick, ~3% max rel err
            on ~40% of chunks -> ~1.1e-2 end-to-end vs 2e-2 budget)
  PV flipped: for each 128-column subtile j: po[:, j, :] [128q, 65] +=
    P^T[c, jslice].T @ [V|1]_c  (stationary P^T, moving V: 65-cycle matmuls,
    full 128-partition output occupancy, no back-transposes needed).
  finalize (DVE): rc = 1/po[:, :, 64]; fin = po[:, :, 0:64]*rc -> bf16;
  DMA out [128, 8, 64] (1KB/partition contiguous); host un-permutes q.

Fully-masked batches (valid_len == 0) are patched on host to the
reference's uniform-softmax value.
"""

import numpy as np
import ml_dtypes

BF16 = ml_dtypes.bfloat16

P = 128          # partitions / k-chunk size
D = 64           # head dim
QL = 1024        # query length
KL = 1024        # key length
NB = 8           # batches
NH = 16          # heads per batch
NCORES = 8
HPC = 16         # heads per core
NCHUNK = KL // P # 8 k-chunks
NEG = -1.0e6

G = 0.125 * np.log2(np.e) * 128.0        # Q pre-scale (Schraudolph exponent)
SCALE_ACT = float(0.125 / G)             # ACT: exp(y0*SCALE_ACT + mask)
MBIAS = float(np.log2(np.e) * 128.0)     # mask multiplier for DVE bias
B0 = 16248.5                             # Schraudolph magic (nearest rounding)
CLAMP = -32000.0                         # masked lanes -> int16 -> bf16 ~ -0

# exp engine schedule: 'A' = ACT exact exp, 'D' = DVE Schraudolph
PATTERN = "ADAADAAD"
_WARMUP = 8
IO_BUFS = 3      # qk/v DMA prefetch depth
PT_BUFS = 2      # P^T tiles in flight (heads)
PACE_DELAY = 1   # chunks of delay before PV groups start in an iteration
POSITIONAL = True  # chunk 0 of each head -> ACT (DVE busy with prev fin)


def _split_excess_waits(nc, max_waits=1):
    """This walrus (gen3) accepts only one sync-wait per instruction, but Tile
    emits up to 2 on compute ops and 5+ on the kernel-tail drain. Hoist excess
    on_wait entries onto fresh InstEventSemaphore ops on the same engine,
    inserted immediately before the offending instruction (same semantics:
    the engine stalls on each wait sequentially)."""
    import bass_rust
    import concourse.mybir as mybir

    n_split = 0
    for func in nc.m.functions:
        for block in func.blocks:
            out = []
            changed = False
            for inst in block.instructions:
                si = getattr(inst, "sync_info", None)
                waits = list(si.on_wait) if si is not None else []
                if len(waits) > max_waits:
                    changed = True
                    for w in waits[:-max_waits]:
                        n_split += 1
                        out.append(
                            mybir.InstEventSemaphore(
                                name=f"waitsplit_{n_split}_{inst.name}",
                                engine=inst.engine,
                                ins=[],
                                outs=[],
                                sync_info=bass_rust.SyncInfo(
                                    on_wait=[w], on_update=[]
                                ),
                            )
                        )
                    inst.sync_info = bass_rust.SyncInfo(
                        on_wait=waits[-max_waits:], on_update=list(si.on_update)
                    )
                out.append(inst)
            if changed:
                block.instructions = out
    return n_split


def _build(nc_chunks=None):
    import concourse.bass as bass
    import concourse.mybir as mybir
    from concourse.tile import TileContext

    if nc_chunks is None:
        nc_chunks = [NCHUNK] * NB

    f32 = mybir.dt.float32
    bf16 = mybir.dt.bfloat16
    i16 = mybir.dt.int16
    Exp = mybir.ActivationFunctionType.Exp
    Add = mybir.AluOpType.add
    Max = mybir.AluOpType.max

    nc = bass.Bass(trn_type="TRN2")
    qkd = nc.dram_tensor("qk", [HPC, D, QL + KL], bf16, kind="ExternalInput")
    vd = nc.dram_tensor("v", [HPC, P, NCHUNK, D + 1], bf16, kind="ExternalInput")
    md = nc.dram_tensor("mask", [P, 2 * NB * NCHUNK], f32, kind="ExternalInput")
    od = nc.dram_tensor("out", [HPC, P, NCHUNK, D], bf16, kind="ExternalOutput")

    gidx = [0]  # global exp-chunk counter for the A/D schedule

    with TileContext(nc) as tc:
        with (
            tc.tile_pool(name="consts", bufs=1) as consts,
            tc.tile_pool(name="io", bufs=IO_BUFS) as io,
            tc.tile_pool(name="pt", bufs=PT_BUFS) as ptp,
            tc.tile_pool(name="fin", bufs=2) as finp,
            tc.tile_pool(name="rc", bufs=2) as rcp,
            tc.tile_pool(name="ps_s", bufs=3, space="PSUM") as ps_s,
            tc.tile_pool(name="ps_o", bufs=1, space="PSUM") as ps_o,
        ):
            # preamble ordering matters: the Pool queue runs the tiny
            # memsets BEFORE the mask SWDGE load (so the PE warmups start
            # early), and the first head's qk DMA goes on the ACT ring
            # BEFORE the exp-table priming activation (so it isn't stuck
            # behind the table load)
            mask_sb = consts.tile([P, 2, NB, NCHUNK], f32)
            scratch = consts.tile([1, 1], f32)
            warm_sb = consts.tile([P, 256], bf16)
            nc.gpsimd.memset(scratch, 0.0)
            nc.gpsimd.memset(warm_sb, 0.0)
            nc.gpsimd.dma_start(
                out=mask_sb, in_=md.rearrange("p (t b c) -> p t b c", t=2, b=NB)
            )

            def emit_front(h, first=False):
                b = h // 2
                nck = nc_chunks[b]
                qk = io.tile([D, QL + KL], bf16, tag="qk")
                qt = qk[:, 0:QL]
                kt = qk[:, QL : QL + KL]
                if first:
                    # the first score matmul needs qt halves + kt chunk 0;
                    # one contiguous load covers qt plus kt chunk 0, a second
                    # brings the rest (the HWDGE processes descriptors
                    # serially, so fewer/earlier descriptors win)
                    nc.scalar.dma_start(
                        out=qk[:, 0 : QL + P], in_=qkd[h][:, 0 : QL + P]
                    )
                    nc.sync.dma_start(
                        out=qk[:, QL + P :], in_=qkd[h][:, QL + P :]
                    )
                else:
                    nc.sync.dma_start(
                        out=qk[:, 0 : QL + nck * P],
                        in_=qkd[h][:, 0 : QL + nck * P],
                    )
                v1 = io.tile([P, NCHUNK, D + 1], bf16, tag="v")
                nc.sync.dma_start(
                    out=v1[:, 0:nck, :], in_=vd[h][:, 0:nck, :]
                )
                return qt, kt, v1

            def emit_score_chunk(h, state, pt, c, force_kind=None):
                b = h // 2
                qt, kt, v1 = state
                ps = ps_s.tile([P, QL], f32, tag="s")
                nc.tensor.matmul(
                    ps[:, 0:512],
                    kt[:, c * P : (c + 1) * P],
                    qt[:, 0:512],
                    start=True, stop=True,
                )
                nc.tensor.matmul(
                    ps[:, 512:QL],
                    kt[:, c * P : (c + 1) * P],
                    qt[:, 512:QL],
                    start=True, stop=True,
                )
                if force_kind is not None:
                    kind = force_kind
                elif POSITIONAL:
                    # DVE is mid-finalize when a head's first chunks arrive
                    if c == 0:
                        kind = "A"
                    else:
                        kind = "DA"[gidx[0] % 2]
                        gidx[0] += 1
                else:
                    kind = PATTERN[gidx[0] % len(PATTERN)]
                    gidx[0] += 1
                if kind == "A":
                    nc.scalar.activation(
                        pt[:, c, :], ps, Exp,
                        bias=mask_sb[:, 0, b, c : c + 1], scale=SCALE_ACT,
                    )
                else:
                    pt_i16 = pt.bitcast(i16)
                    nc.vector.tensor_scalar(
                        pt_i16[:, c, :], ps,
                        mask_sb[:, 1, b, c : c + 1], CLAMP,
                        op0=Add, op1=Max,
                    )

            def emit_pv_group(h, po, pt, v1, j, nck):
                # one output subtile j: consecutive accumulating matmuls
                # (interleaved PSUM accumulation groups don't accumulate
                # correctly, so keep each group's matmuls back-to-back)
                for c in range(nck):
                    nc.tensor.matmul(
                        po[:, j, 0 : D + 1],
                        pt[:, c, j * P : (j + 1) * P],
                        v1[:, c, :],
                        start=(c == 0), stop=(c == nck - 1),
                    )

            def emit_fin(h, po):
                rc = rcp.tile([P, NCHUNK], f32, tag="rc")
                nc.vector.reciprocal(rc, po[:, :, D : D + 1])
                fin = finp.tile([P, NCHUNK, D], bf16, tag="fin")
                nc.vector.tensor_mul(
                    fin,
                    po[:, :, 0:D],
                    rc[:, :, None].broadcast_to([P, NCHUNK, D]),
                )
                nc.sync.dma_start(out=od[h], in_=fin)

            def emit_iter(cur, prev, last=False):
                """cur = (h, state): scores+exp; prev = (h, state, pt):
                PV + finalize, interleaved chunk-wise on the PE. PV group
                emission is delayed by one chunk so the previous head's
                finalize (which frees the single po buffer) clears first."""
                pt_cur = None
                if cur is not None:
                    h, state = cur
                    nck = nc_chunks[h // 2]
                    pt_cur = ptp.tile([P, NCHUNK, QL], bf16, tag="pt")
                po = None
                if prev is not None:
                    ph, pstate, ppt = prev
                    pnck = nc_chunks[ph // 2]
                    # [P, 8, 128] f32: 512B-aligned j-subtiles (a matmul
                    # output must not cross a 2KB PSUM bank boundary). The
                    # tail iteration takes a ps_s slot instead: no score
                    # tiles compete then, and it skips the po-buffer WAR
                    # against the previous finalize.
                    if last:
                        po = ps_s.tile([P, NCHUNK, P], f32, tag="s")
                    else:
                        po = ps_o.tile([P, NCHUNK, P], f32, tag="o")
                n_s = nck if cur is not None else 0
                steps = max(n_s, 1)
                for c in range(steps):
                    if c < n_s:
                        # the final head's exps alternate engines so they
                        # run in parallel rather than queueing on one
                        emit_score_chunk(
                            h, state, pt_cur, c,
                            force_kind=("DA"[c % 2] if last else None),
                        )
                    if prev is not None:
                        dd = PACE_DELAY
                        if steps <= dd:
                            jlo, jhi = 0, NCHUNK
                        else:
                            jlo = NCHUNK * max(c - dd, 0) // (steps - dd)
                            jhi = NCHUNK * max(c + 1 - dd, 0) // (steps - dd)
                        for j in range(jlo, jhi):
                            emit_pv_group(ph, po, ppt, pstate[2], j, pnck)
                if prev is not None:
                    emit_fin(ph, po)
                return pt_cur

            # Interleave big and small heads so engine loads stay balanced;
            # end with the smallest head (shortest un-hidden tail).
            by_size = sorted(range(HPC), key=lambda h: -nc_chunks[h // 2])
            big, small = by_size[: HPC // 2], by_size[HPC // 2 :]
            order = [h for pair in zip(big, small) for h in pair]

            fronts = {}
            fronts[order[0]] = emit_front(order[0], first=True)
            # prime the ScalarE exp table load (~1.3us) and the PE p-state
            # ramp while the first qk DMA is in flight
            nc.scalar.activation(scratch, scratch, Exp)
            warm = ps_s.tile([1, 256], f32, tag="s")
            for _ in range(_WARMUP):
                nc.tensor.matmul(
                    warm, warm_sb[:, 0:1], warm_sb[:, 0:256],
                    start=True, stop=True,
                )
            pending = None   # (h, state) awaiting scores
            prev = None      # (h, state, pt) awaiting pv+fin
            for i, h in enumerate(order):
                if i > 0:
                    fronts[h] = emit_front(h)
                if pending is not None:
                    pt = emit_iter(pending, prev)
                    prev = (pending[0], pending[1], pt)
                pending = (h, fronts[h])
            pt = emit_iter(pending, prev, last=True)
            prev = (pending[0], pending[1], pt)
            emit_iter(None, prev, last=True)
    _split_excess_waits(nc)
    return nc


_CACHE = {}


def _get_nc(key, nc_chunks):
    if key not in _CACHE:
        _CACHE[key] = _build(nc_chunks)
    return _CACHE[key]


def _core_head_idx(c):
    return [b * NH + 2 * c + j for b in range(NB) for j in range(2)]


def _run(in_maps, nc, trace=False):
    from concourse.bass_utils import run_bass_kernel_spmd

    return run_bass_kernel_spmd(
        nc, in_maps, core_ids=list(range(NCORES)), trace=trace
    )


# column cblk*128 + p holds query q = (cblk//4)*512 + 4p + (cblk%4)
_COLQ = np.empty(QL, np.int64)
for _cb in range(8):
    for _p in range(P):
        _COLQ[_cb * P + _p] = (_cb // 4) * 512 + 4 * _p + (_cb % 4)
# output index: od[p, cblk, :] -> q = _COLQ[cblk*128 + p]
_OUTQ = np.empty(QL, np.int64)
for _p in range(P):
    for _cb in range(8):
        _OUTQ[_p * 8 + _cb] = _COLQ[_cb * P + _p]


def _prepare(queries, keys, values, valid_lens):
    queries = np.asarray(queries, np.float32)
    keys = np.asarray(keys, np.float32)
    values = np.asarray(values, np.float32)
    vl = np.asarray(valid_lens).astype(np.int64)
    maskv = np.where(
        np.arange(KL)[None, :] >= vl[:, None], np.float32(NEG), np.float32(0.0)
    ).astype(np.float32)  # [NB, KL]
    # [p, b, c] = mask[b, c*128 + p]
    m_pbc = maskv.reshape(NB, NCHUNK, P).transpose(2, 0, 1)
    mask_dev = np.empty((P, 2, NB, NCHUNK), np.float32)
    mask_dev[:, 0] = m_pbc
    mask_dev[:, 1] = m_pbc * np.float32(MBIAS) + np.float32(B0)
    mask_dev = np.ascontiguousarray(
        mask_dev.reshape(P, 2 * NB * NCHUNK)
    )
    nc_chunks = [max(1, int(min(NCHUNK, (int(v) + P - 1) // P))) for v in vl]
    bh = queries.shape[0]
    # [Q^T | K^T] combined, bf16; Q pre-scaled, 4-paired column order
    qkp = np.empty((bh, D, QL + KL), BF16)
    qkp[:, :, 0:QL] = (queries[:, _COLQ, :] * np.float32(G)).transpose(0, 2, 1)
    qkp[:, :, QL:] = keys.transpose(0, 2, 1)
    # V + ones column, partition-major: [bh, 128, 8, 65]
    v1 = np.concatenate(
        [values, np.ones((bh, KL, 1), np.float32)], axis=-1
    )
    v1p = np.ascontiguousarray(
        v1.reshape(bh, NCHUNK, P, D + 1).transpose(0, 2, 1, 3)
    ).astype(BF16)
    in_maps = []
    for c in range(NCORES):
        idx = _core_head_idx(c)
        in_maps.append(
            {
                "qk": qkp[idx],
                "v": v1p[idx],
                "mask": mask_dev,
            }
        )
    return in_maps, nc_chunks, vl


def _gather(results, values, vl):
    out = np.empty((NB * NH, QL, D), np.float32)
    for c in range(NCORES):
        o = np.asarray(results[c]["out"]).astype(np.float32)  # [16,128,8,64]
        out[_core_head_idx(c)] = _unpermute(o)
    # fully-masked batches: reference softmax(-1e6 * ones) is uniform
    for b in range(NB):
        if vl[b] == 0:
            for hh in range(NH):
                bhh = b * NH + hh
                out[bhh] = np.asarray(values[bhh], np.float32).mean(
                    axis=0, keepdims=True
                )
    return out


def _unpermute(o):
    # o [HPC, 128, 8, 64] -> [HPC, QL, D] with q = _OUTQ[p*8+c]
    flat = o.reshape(HPC, QL, D)
    res = np.empty_like(flat)
    res[:, _OUTQ] = flat
    return res


def kernel(queries, keys, values, valid_lens):
    in_maps, nc_chunks, vl = _prepare(queries, keys, values, valid_lens)
    nc = _get_nc(tuple(nc_chunks), nc_chunks)
    res = _run(in_maps, nc)
    return _gather(res.results, values, vl)


# revision 25
# speedup vs baseline: 1.0665x; 1.0015x over previous
"""Masked dot-product attention on 8 Trainium2 NeuronCores.

Problem shapes (hardcoded): queries/keys/values [128, 1024, 64] f32,
valid_lens [8] int (per-batch key valid length; BH = 8 batches x 16 heads).

Sharding: core c handles heads {b*16 + 2c, b*16 + 2c + 1} for all batches b
(16 heads/core, every batch present on every core -> uniform work, one
compiled program serves all cores).

Layout (host prep, all attention math on device):
  - Q^T [BH, 64, 1024] bf16, PRE-SCALED by G = 0.125*log2(e)*128 so device
    scores arrive as y0 = s*G (the Schraudolph exponent scale). Columns are
    4-way-paired: column cblk*128 + p holds query q = (cblk//4)*512 + 4p +
    (cblk%4), so each PV output subtile maps to >=512B-contiguous DMA runs.
  - K^T [BH, 64, 1024] bf16 natural order (valid-len truncation at 128-chunk
    granularity).
  - V augmented with a ones column (softmax-denominator trick), partition-
    major: [BH, 128, 8, 65] bf16.
  - mask biases [128, 2, b, c]: variant 0 for ACT (0 / -1e6), variant 1 for
    DVE (m*log2e*128 + B0, i.e. the Schraudolph integer bias).

Per-head device pipeline (scores transposed, S^T[k, q]):
  per k-chunk c (only chunks below the batch's valid_len):
    y0^T[c] [128, 1024] = K^T_c.T @ Q^T_scaled      (PSUM f32, 2 bf16 matmuls)
    exp split by static schedule:
      ACT: P^T = exp(y0*SCALE_ACT + mask)  -> bf16  (exact)
      DVE: P^T = bitcast_bf16(int16(max(y0 + maskbias, -32000)))
           (one tensor_scalar; Schraudolph exp2 bit-trick, ~3% max rel err
            on ~40% of chunks -> ~1.1e-2 end-to-end vs 2e-2 budget)
  PV flipped: for each 128-column subtile j: po[:, j, :] [128q, 65] +=
    P^T[c, jslice].T @ [V|1]_c  (stationary P^T, moving V: 65-cycle matmuls,
    full 128-partition output occupancy, no back-transposes needed).
  finalize (DVE): rc = 1/po[:, :, 64]; fin = po[:, :, 0:64]*rc -> bf16;
  DMA out [128, 8, 64] (1KB/partition contiguous); host un-permutes q.

Fully-masked batches (valid_len == 0) are patched on host to the
reference's uniform-softmax value.
"""

import numpy as np
import ml_dtypes

BF16 = ml_dtypes.bfloat16

P = 128          # partitions / k-chunk size
D = 64           # head dim
QL = 1024        # query length
KL = 1024        # key length
NB = 8           # batches
NH = 16          # heads per batch
NCORES = 8
HPC = 16         # heads per core
NCHUNK = KL // P # 8 k-chunks
NEG = -1.0e6

G = 0.125 * np.log2(np.e) * 128.0        # Q pre-scale (Schraudolph exponent)
SCALE_ACT = float(0.125 / G)             # ACT: exp(y0*SCALE_ACT + mask)
MBIAS = float(np.log2(np.e) * 128.0)     # mask multiplier for DVE bias
B0 = 16248.5                             # Schraudolph magic (nearest rounding)
CLAMP = -32000.0                         # masked lanes -> int16 -> bf16 ~ -0

# exp engine schedule: 'A' = ACT exact exp, 'D' = DVE Schraudolph
PATTERN = "ADAADAAD"
_WARMUP = 8
IO_BUFS = 3      # qk/v DMA prefetch depth
PT_BUFS = 2      # P^T tiles in flight (heads)
PACE_DELAY = 1   # chunks of delay before PV groups start in an iteration
POSITIONAL = True  # chunk 0 of each head -> ACT (DVE busy with prev fin)
OUT_ON_POOL = False  # out-DMA on the idle SWDGE queue (unblocks SP prefetch)
FRONT_LEAD = 2   # how many heads ahead input DMAs are issued


def _split_excess_waits(nc, max_waits=1):
    """This walrus (gen3) accepts only one sync-wait per instruction, but Tile
    emits up to 2 on compute ops and 5+ on the kernel-tail drain. Hoist excess
    on_wait entries onto fresh InstEventSemaphore ops on the same engine,
    inserted immediately before the offending instruction (same semantics:
    the engine stalls on each wait sequentially)."""
    import bass_rust
    import concourse.mybir as mybir

    n_split = 0
    for func in nc.m.functions:
        for block in func.blocks:
            out = []
            changed = False
            for inst in block.instructions:
                si = getattr(inst, "sync_info", None)
                waits = list(si.on_wait) if si is not None else []
                if len(waits) > max_waits:
                    changed = True
                    for w in waits[:-max_waits]:
                        n_split += 1
                        out.append(
                            mybir.InstEventSemaphore(
                                name=f"waitsplit_{n_split}_{inst.name}",
                                engine=inst.engine,
                                ins=[],
                                outs=[],
                                sync_info=bass_rust.SyncInfo(
                                    on_wait=[w], on_update=[]
                                ),
                            )
                        )
                    inst.sync_info = bass_rust.SyncInfo(
                        on_wait=waits[-max_waits:], on_update=list(si.on_update)
                    )
                out.append(inst)
            if changed:
                block.instructions = out
    return n_split


def _build(nc_chunks=None):
    import concourse.bass as bass
    import concourse.mybir as mybir
    from concourse.tile import TileContext

    if nc_chunks is None:
        nc_chunks = [NCHUNK] * NB

    f32 = mybir.dt.float32
    bf16 = mybir.dt.bfloat16
    i16 = mybir.dt.int16
    Exp = mybir.ActivationFunctionType.Exp
    Add = mybir.AluOpType.add
    Max = mybir.AluOpType.max

    nc = bass.Bass(trn_type="TRN2")
    qkd = nc.dram_tensor("qk", [HPC, D, QL + KL], bf16, kind="ExternalInput")
    vd = nc.dram_tensor("v", [HPC, P, NCHUNK, D + 1], bf16, kind="ExternalInput")
    md = nc.dram_tensor("mask", [P, 2 * NB * NCHUNK], f32, kind="ExternalInput")
    od = nc.dram_tensor("out", [HPC, P, NCHUNK, D], bf16, kind="ExternalOutput")

    gidx = [0]  # global exp-chunk counter for the A/D schedule

    with TileContext(nc) as tc:
        with (
            tc.tile_pool(name="consts", bufs=1) as consts,
            tc.tile_pool(name="io", bufs=IO_BUFS) as io,
            tc.tile_pool(name="pt", bufs=PT_BUFS) as ptp,
            tc.tile_pool(name="fin", bufs=2) as finp,
            tc.tile_pool(name="rc", bufs=2) as rcp,
            tc.tile_pool(name="ps_s", bufs=3, space="PSUM") as ps_s,
            tc.tile_pool(name="ps_o", bufs=1, space="PSUM") as ps_o,
        ):
            # preamble ordering matters: the Pool queue runs the tiny
            # memsets BEFORE the mask SWDGE load (so the PE warmups start
            # early), and the first head's qk DMA goes on the ACT ring
            # BEFORE the exp-table priming activation (so it isn't stuck
            # behind the table load)
            mask_sb = consts.tile([P, 2, NB, NCHUNK], f32)
            scratch = consts.tile([1, 1], f32)
            warm_sb = consts.tile([P, 256], bf16)
            nc.gpsimd.memset(scratch, 0.0)
            nc.gpsimd.memset(warm_sb, 0.0)
            nc.gpsimd.dma_start(
                out=mask_sb, in_=md.rearrange("p (t b c) -> p t b c", t=2, b=NB)
            )

            def emit_front(h, first=False):
                b = h // 2
                nck = nc_chunks[b]
                qk = io.tile([D, QL + KL], bf16, tag="qk")
                qt = qk[:, 0:QL]
                kt = qk[:, QL : QL + KL]
                if first:
                    # the first score matmul needs qt halves + kt chunk 0;
                    # one contiguous load covers qt plus kt chunk 0, a second
                    # brings the rest (the HWDGE processes descriptors
                    # serially, so fewer/earlier descriptors win)
                    nc.scalar.dma_start(
                        out=qk[:, 0 : QL + P], in_=qkd[h][:, 0 : QL + P]
                    )
                    nc.sync.dma_start(
                        out=qk[:, QL + P :], in_=qkd[h][:, QL + P :]
                    )
                else:
                    nc.sync.dma_start(
                        out=qk[:, 0 : QL + nck * P],
                        in_=qkd[h][:, 0 : QL + nck * P],
                    )
                v1 = io.tile([P, NCHUNK, D + 1], bf16, tag="v")
                nc.sync.dma_start(
                    out=v1[:, 0:nck, :], in_=vd[h][:, 0:nck, :]
                )
                return qt, kt, v1

            def emit_score_chunk(h, state, pt, c, force_kind=None):
                b = h // 2
                qt, kt, v1 = state
                ps = ps_s.tile([P, QL], f32, tag="s")
                nc.tensor.matmul(
                    ps[:, 0:512],
                    kt[:, c * P : (c + 1) * P],
                    qt[:, 0:512],
                    start=True, stop=True,
                )
                nc.tensor.matmul(
                    ps[:, 512:QL],
                    kt[:, c * P : (c + 1) * P],
                    qt[:, 512:QL],
                    start=True, stop=True,
                )
                if force_kind is not None:
                    kind = force_kind
                elif POSITIONAL:
                    # DVE is mid-finalize when a head's first chunks arrive
                    if c == 0:
                        kind = "A"
                    else:
                        kind = "DA"[gidx[0] % 2]
                        gidx[0] += 1
                else:
                    kind = PATTERN[gidx[0] % len(PATTERN)]
                    gidx[0] += 1
                if kind == "A":
                    nc.scalar.activation(
                        pt[:, c, :], ps, Exp,
                        bias=mask_sb[:, 0, b, c : c + 1], scale=SCALE_ACT,
                    )
                else:
                    pt_i16 = pt.bitcast(i16)
                    nc.vector.tensor_scalar(
                        pt_i16[:, c, :], ps,
                        mask_sb[:, 1, b, c : c + 1], CLAMP,
                        op0=Add, op1=Max,
                    )

            def emit_pv_group(h, po, pt, v1, j, nck):
                # one output subtile j: consecutive accumulating matmuls
                # (interleaved PSUM accumulation groups don't accumulate
                # correctly, so keep each group's matmuls back-to-back)
                for c in range(nck):
                    nc.tensor.matmul(
                        po[:, j, 0 : D + 1],
                        pt[:, c, j * P : (j + 1) * P],
                        v1[:, c, :],
                        start=(c == 0), stop=(c == nck - 1),
                    )

            def emit_fin(h, po):
                rc = rcp.tile([P, NCHUNK], f32, tag="rc")
                nc.vector.reciprocal(rc, po[:, :, D : D + 1])
                fin = finp.tile([P, NCHUNK, D], bf16, tag="fin")
                nc.vector.tensor_mul(
                    fin,
                    po[:, :, 0:D],
                    rc[:, :, None].broadcast_to([P, NCHUNK, D]),
                )
                if OUT_ON_POOL:
                    nc.gpsimd.dma_start(out=od[h], in_=fin)
                else:
                    nc.sync.dma_start(out=od[h], in_=fin)

            def emit_iter(cur, prev, last=False):
                """cur = (h, state): scores+exp; prev = (h, state, pt):
                PV + finalize, interleaved chunk-wise on the PE. PV group
                emission is delayed by one chunk so the previous head's
                finalize (which frees the single po buffer) clears first."""
                pt_cur = None
                if cur is not None:
                    h, state = cur
                    nck = nc_chunks[h // 2]
                    pt_cur = ptp.tile([P, NCHUNK, QL], bf16, tag="pt")
                po = None
                if prev is not None:
                    ph, pstate, ppt = prev
                    pnck = nc_chunks[ph // 2]
                    # [P, 8, 128] f32: 512B-aligned j-subtiles (a matmul
                    # output must not cross a 2KB PSUM bank boundary). The
                    # tail iteration takes a ps_s slot instead: no score
                    # tiles compete then, and it skips the po-buffer WAR
                    # against the previous finalize.
                    if last:
                        po = ps_s.tile([P, NCHUNK, P], f32, tag="s")
                    else:
                        po = ps_o.tile([P, NCHUNK, P], f32, tag="o")
                n_s = nck if cur is not None else 0
                steps = max(n_s, 1)
                for c in range(steps):
                    if c < n_s:
                        # the final head's exps alternate engines so they
                        # run in parallel rather than queueing on one
                        emit_score_chunk(
                            h, state, pt_cur, c,
                            force_kind=("DA"[c % 2] if last else None),
                        )
                    if prev is not None:
                        dd = PACE_DELAY
                        if steps <= dd:
                            jlo, jhi = 0, NCHUNK
                        else:
                            jlo = NCHUNK * max(c - dd, 0) // (steps - dd)
                            jhi = NCHUNK * max(c + 1 - dd, 0) // (steps - dd)
                        for j in range(jlo, jhi):
                            emit_pv_group(ph, po, ppt, pstate[2], j, pnck)
                if prev is not None:
                    emit_fin(ph, po)
                return pt_cur

            # Interleave big and small heads so engine loads stay balanced;
            # end with the smallest head (shortest un-hidden tail).
            by_size = sorted(range(HPC), key=lambda h: -nc_chunks[h // 2])
            big, small = by_size[: HPC // 2], by_size[HPC // 2 :]
            order = [h for pair in zip(big, small) for h in pair]

            fronts = {}
            fronts[order[0]] = emit_front(order[0], first=True)
            # prime the ScalarE exp table load (~1.3us) and the PE p-state
            # ramp while the first qk DMA is in flight
            nc.scalar.activation(scratch, scratch, Exp)
            warm = ps_s.tile([1, 256], f32, tag="s")
            for _ in range(_WARMUP):
                nc.tensor.matmul(
                    warm, warm_sb[:, 0:1], warm_sb[:, 0:256],
                    start=True, stop=True,
                )
            for j in range(1, 1 + FRONT_LEAD):
                fronts[order[j]] = emit_front(order[j])
            pending = None   # (h, state) awaiting scores
            prev = None      # (h, state, pt) awaiting pv+fin
            for i, h in enumerate(order):
                if i + FRONT_LEAD < len(order):
                    fronts[order[i + FRONT_LEAD]] = emit_front(
                        order[i + FRONT_LEAD]
                    )
                if pending is not None:
                    pt = emit_iter(pending, prev)
                    prev = (pending[0], pending[1], pt)
                pending = (h, fronts[h])
            pt = emit_iter(pending, prev, last=True)
            prev = (pending[0], pending[1], pt)
            emit_iter(None, prev, last=True)
    _split_excess_waits(nc)
    return nc


_CACHE = {}


def _get_nc(key, nc_chunks):
    if key not in _CACHE:
        _CACHE[key] = _build(nc_chunks)
    return _CACHE[key]


def _core_head_idx(c):
    return [b * NH + 2 * c + j for b in range(NB) for j in range(2)]


def _run(in_maps, nc, trace=False):
    from concourse.bass_utils import run_bass_kernel_spmd

    return run_bass_kernel_spmd(
        nc, in_maps, core_ids=list(range(NCORES)), trace=trace
    )


# column cblk*128 + p holds query q = (cblk//4)*512 + 4p + (cblk%4)
_COLQ = np.empty(QL, np.int64)
for _cb in range(8):
    for _p in range(P):
        _COLQ[_cb * P + _p] = (_cb // 4) * 512 + 4 * _p + (_cb % 4)
# output index: od[p, cblk, :] -> q = _COLQ[cblk*128 + p]
_OUTQ = np.empty(QL, np.int64)
for _p in range(P):
    for _cb in range(8):
        _OUTQ[_p * 8 + _cb] = _COLQ[_cb * P + _p]


def _prepare(queries, keys, values, valid_lens):
    queries = np.asarray(queries, np.float32)
    keys = np.asarray(keys, np.float32)
    values = np.asarray(values, np.float32)
    vl = np.asarray(valid_lens).astype(np.int64)
    maskv = np.where(
        np.arange(KL)[None, :] >= vl[:, None], np.float32(NEG), np.float32(0.0)
    ).astype(np.float32)  # [NB, KL]
    # [p, b, c] = mask[b, c*128 + p]
    m_pbc = maskv.reshape(NB, NCHUNK, P).transpose(2, 0, 1)
    mask_dev = np.empty((P, 2, NB, NCHUNK), np.float32)
    mask_dev[:, 0] = m_pbc
    mask_dev[:, 1] = m_pbc * np.float32(MBIAS) + np.float32(B0)
    mask_dev = np.ascontiguousarray(
        mask_dev.reshape(P, 2 * NB * NCHUNK)
    )
    nc_chunks = [max(1, int(min(NCHUNK, (int(v) + P - 1) // P))) for v in vl]
    bh = queries.shape[0]
    # [Q^T | K^T] combined, bf16; Q pre-scaled, 4-paired column order
    qkp = np.empty((bh, D, QL + KL), BF16)
    qkp[:, :, 0:QL] = (queries[:, _COLQ, :] * np.float32(G)).transpose(0, 2, 1)
    qkp[:, :, QL:] = keys.transpose(0, 2, 1)
    # V + ones column, partition-major: [bh, 128, 8, 65]
    v1 = np.concatenate(
        [values, np.ones((bh, KL, 1), np.float32)], axis=-1
    )
    v1p = np.ascontiguousarray(
        v1.reshape(bh, NCHUNK, P, D + 1).transpose(0, 2, 1, 3)
    ).astype(BF16)
    in_maps = []
    for c in range(NCORES):
        idx = _core_head_idx(c)
        in_maps.append(
            {
                "qk": qkp[idx],
                "v": v1p[idx],
                "mask": mask_dev,
            }
        )
    return in_maps, nc_chunks, vl


def _gather(results, values, vl):
    out = np.empty((NB * NH, QL, D), np.float32)
    for c in range(NCORES):
        o = np.asarray(results[c]["out"]).astype(np.float32)  # [16,128,8,64]
        out[_core_head_idx(c)] = _unpermute(o)
    # fully-masked batches: reference softmax(-1e6 * ones) is uniform
    for b in range(NB):
        if vl[b] == 0:
            for hh in range(NH):
                bhh = b * NH + hh
                out[bhh] = np.asarray(values[bhh], np.float32).mean(
                    axis=0, keepdims=True
                )
    return out


def _unpermute(o):
    # o [HPC, 128, 8, 64] -> [HPC, QL, D] with q = _OUTQ[p*8+c]
    flat = o.reshape(HPC, QL, D)
    res = np.empty_like(flat)
    res[:, _OUTQ] = flat
    return res


def kernel(queries, keys, values, valid_lens):
    in_maps, nc_chunks, vl = _prepare(queries, keys, values, valid_lens)
    nc = _get_nc(tuple(nc_chunks), nc_chunks)
    res = _run(in_maps, nc)
    return _gather(res.results, values, vl)
